# revision 32
# baseline (speedup 1.0000x reference)
"""Multi-head self-attention on 8 TRN2 NeuronCores.

Sharding: core c -> (batch b = c//2, head-half g = c%2, i.e. 8 of 16 heads).
Each core computes qkv-proj + attention + out-proj partial for its 8 heads;
host sums the two partials per batch and adds b_out.

Design (v2):
- stage1 q,k,v projections in fp16 (1 cyc/row), q/k results quantized to
  fp8e4 on the PSUM->SBUF copy (bias added via per-partition tensor_scalar).
- scores as zero-padded DoubleRow fp8 matmuls (0.5 cyc/row): operands
  [64, 2, *] with the i=1 plane zeroed; out tile [128 k-pos, 512 q-pos].
- exp on ACT (scale=0.125 applied in the activation), fp16 out.
- ctx computed transposed: out [128 q, 65] with lhsT = exp tile (stationary)
  and rhs = V' [128 k, 65] whose 65th column is ones -> denominator lands in
  out[:, 64] = per-partition scalar. Normalization + V-bias is then a single
  scalar_tensor_tensor (mult, add) per (head, q-tile).
- ctx^T via PE transpose (fp16, identity rhs) packing head pairs into
  [128, 128] PSUM tiles; out-projection over the packed [d, q] layout,
  fp16 output DMA'd per tile; host sums core pairs + b_out.
- software pipelining: unit (pair, qc) emits its 32 score matmuls + 16 exps,
  then the previous unit's ctx/norm/transpose tail, then next-pair stage1
  or out-projection work, keeping ACT (the bottleneck) saturated.
"""
import sys
sys.path.insert(0, '/opt/trn_rl_repo')

import numpy as np

import concourse.bass as bass
import concourse.mybir as mybir
import concourse.tile as tile
from concourse import bacc

F32 = mybir.dt.float32
F16 = mybir.dt.float16
F8E4 = mybir.dt.float8e4
I16 = mybir.dt.int16
DR = mybir.MatmulPerfMode.DoubleRow
Exp = mybir.ActivationFunctionType.Exp
MULT = mybir.AluOpType.mult
ADD = mybir.AluOpType.add

B, S, D = 4, 2048, 1024
H, HD = 16, 64
N_CORES = 8
NSK = S // 128            # 16 k-chunks of 128
NQT = S // 128            # 16 q-tiles of 128
SCALE = 0.125             # 1/sqrt(HD)

# Schraudolph exp for DVE/GPSIMD offload: fp16 bits = trunc(ALPHA*s + BETA)
# approximates exp(s * SCALE) to ~3% max rel error (error cancels partially
# in the softmax ratio). ALPHA = 1024*log2(e)*SCALE.
ALPHA = 184.6649652337873
BETA = 15316.431477991726

# per-unit exp engine assignment (16 slots): A=ACT native exp,
# D=DVE Schraudolph, P=GPSIMD Schraudolph
# 32 slots, 16A/9D/7P: balances ACT/DVE/GPSIMD busy at ~155-165us each
EXP_PATTERN = "ADPAADPAADPAADAPADPAADPAADPAADAD"

# stage1 projections as fp8 DoubleRow matmuls (halves PE cost of each).
# Weights are pre-scaled by WSCL on the host so W_in/W_out values
# (~U(-1/32,1/32)) sit mid-range in fp8e4m3 instead of subnormal;
# compensated by 1/WSCL on the PSUM->SBUF copies.
QK_DR = True
V_DR = True
WSCL = 64.0


def build_nc(skip_tail=False, skip_exp=False, sc_bufs=4, skip_v=False, fake_in=False, debug=False):
    nc = bacc.Bacc(None, target_bir_lowering=False)

    if QK_DR or V_DR:
        x8_d = nc.dram_tensor("x8", [64, 2, 8, S], F8E4, kind="ExternalInput")
    if not (QK_DR and V_DR):
        x16_d = nc.dram_tensor("x16", [128, 8, S], F16, kind="ExternalInput")
    if QK_DR:
        wqk8_d = nc.dram_tensor("wqk8", [64, 2, 8, 1024], F8E4,
                                kind="ExternalInput")
    else:
        wqk_d = nc.dram_tensor("wqk", [128, 8, 1024], F16,
                               kind="ExternalInput")
    if V_DR:
        wv8_d = nc.dram_tensor("wv8", [64, 2, 8, 512], F8E4,
                               kind="ExternalInput")
    else:
        wv_d = nc.dram_tensor("wv", [128, 8, 512], F16, kind="ExternalInput")
    wout_d = nc.dram_tensor("wout", [128, 4, D], F16, kind="ExternalInput")
    bqk_d = nc.dram_tensor("bqk", [128, 8], F32, kind="ExternalInput")
    bv_d = nc.dram_tensor("bv", [128, 8, HD], F16, kind="ExternalInput")
    ident_d = nc.dram_tensor("ident", [128, 128], F16, kind="ExternalInput")
    out_d = nc.dram_tensor("out", [S, D], F16, kind="ExternalOutput")
    if debug:
        dbg_qk8 = nc.dram_tensor("dbg_qk8", [128, 2, 2, S], F8E4,
                                 kind="ExternalOutput")
        dbg_expT = nc.dram_tensor("dbg_expT", [128, NSK, 512], F16,
                                  kind="ExternalOutput")
        dbg_ctxT = nc.dram_tensor("dbg_ctxT", [128, 4, S], F16,
                                  kind="ExternalOutput")
        dbg_v = nc.dram_tensor("dbg_v", [128, NSK, 8, HD + 1], F16,
                               kind="ExternalOutput")

    with tile.TileContext(nc) as tc:
        with (
            tc.tile_pool(name="const", bufs=1) as cpool,
            tc.tile_pool(name="expT", bufs=4) as expT_pool,
            tc.tile_pool(name="ctxN", bufs=4) as ctxN_pool,
            tc.tile_pool(name="rcp", bufs=4) as rcp_pool,
            tc.tile_pool(name="osb", bufs=2) as out_pool,
            tc.tile_pool(name="scps", bufs=sc_bufs, space="PSUM") as sc_ps,
            tc.tile_pool(name="ctxps", bufs=2, space="PSUM") as ctx_ps,
            tc.tile_pool(name="shps", bufs=2, space="PSUM") as sh_ps,
        ):
            # ---- constants / persistent tiles (DMAs ordered so the
            # prologue's pair-0 stage1 work can start immediately) ----
            if QK_DR or V_DR:
                x8 = cpool.tile([64, 2, 8, S], F8E4)
            if not (QK_DR and V_DR):
                x16 = cpool.tile([128, 8, S], F16)
            if QK_DR:
                wqk8 = cpool.tile([64, 2, 8, 1024], F8E4)
            else:
                wqk = cpool.tile([128, 8, 1024], F16)
            bqk = cpool.tile([128, 8], F32)
            if V_DR:
                wv8 = cpool.tile([64, 2, 8, 512], F8E4)
            else:
                wv = cpool.tile([128, 8, 512], F16)
            wout = cpool.tile([128, 4, D], F16)
            bv = cpool.tile([128, 8, HD], F16)
            ident = cpool.tile([128, 128], F16)
            # critical-path DMAs on SP, halves sized to unblock the first
            # stage1 half-tile ASAP; bulk loads ride the ACT hwdge queue
            nc.sync.dma_start(bqk[:], bqk_d[:])
            if QK_DR:
                nc.sync.dma_start(wqk8[:, :, :, 512:640], wqk8_d[:, :, :, 512:640])
            else:
                nc.sync.dma_start(wqk[:, :, 512:640], wqk_d[:, :, 512:640])
            if QK_DR or V_DR:
                nc.sync.dma_start(x8[:, :, :, 0:512], x8_d[:, :, :, 0:512])
            if not (QK_DR and V_DR):
                nc.sync.dma_start(x16[:, :, 0:512], x16_d[:, :, 0:512])
            if QK_DR:
                nc.sync.dma_start(wqk8[:, :, :, 0:128], wqk8_d[:, :, :, 0:128])
            else:
                nc.sync.dma_start(wqk[:, :, 0:128], wqk_d[:, :, 0:128])
            for n in range(1, 4):
                if QK_DR or V_DR:
                    nc.sync.dma_start(x8[:, :, :, 512 * n:512 * (n + 1)],
                                      x8_d[:, :, :, 512 * n:512 * (n + 1)])
                if not (QK_DR and V_DR):
                    nc.sync.dma_start(x16[:, :, 512 * n:512 * (n + 1)],
                                      x16_d[:, :, 512 * n:512 * (n + 1)])
            if V_DR:
                nc.sync.dma_start(wv8[:], wv8_d[:])
            else:
                nc.sync.dma_start(wv[:], wv_d[:])
            nc.sync.dma_start(bv[:], bv_d[:])
            nc.sync.dma_start(ident[:], ident_d[:])
            for p_ in range(1, 4):
                if QK_DR:
                    nc.sync.dma_start(
                        wqk8[:, :, :, 512 + 128 * p_:640 + 128 * p_],
                        wqk8_d[:, :, :, 512 + 128 * p_:640 + 128 * p_])
                    nc.sync.dma_start(
                        wqk8[:, :, :, 128 * p_:128 * (p_ + 1)],
                        wqk8_d[:, :, :, 128 * p_:128 * (p_ + 1)])
                else:
                    nc.sync.dma_start(
                        wqk[:, :, 512 + 128 * p_:640 + 128 * p_],
                        wqk_d[:, :, 512 + 128 * p_:640 + 128 * p_])
                    nc.sync.dma_start(wqk[:, :, 128 * p_:128 * (p_ + 1)],
                                      wqk_d[:, :, 128 * p_:128 * (p_ + 1)])
            nc.sync.dma_start(wout[:], wout_d[:])

            # PE p-state warm-up: the cost model charges cold-clock cycles
            # until the PE has been continuously busy for 3us at prep time.
            # Run a dependency-free accumulating matmul chain from t=0 so the
            # real prologue matmuls are prepped at the full 2.4 GHz clock.
            wrm = cpool.tile([128, 512], F16)
            nc.vector.memset(wrm[:], 0.125)
            # preload the ACT exp table at t~0 (1.3us) so the first real
            # exp doesn't eat the table-load latency mid-pipeline
            actwarm = cpool.tile([128, 1], F16)
            nc.scalar.activation(actwarm[:], wrm[:, 0:1], Exp, scale=1.0)
            wps = sh_ps.tile([128, 512], F32, name="wps", tag="sh")
            NWARM = 14
            for i in range(NWARM):
                nc.tensor.matmul(wps[:], wrm[:, 0:128], wrm[:],
                                 start=(i == 0), stop=(i == NWARM - 1))

            # V': [s%128, sk, head, hd+1]; [..,64] = 1.0 for denominators
            v_sb = cpool.tile([128, NSK, 8, HD + 1], F16)
            nc.vector.memset(v_sb[:, :, :, HD], 1.0)

            # q/k fp8 double-buffered (ping-pong by pair parity).
            # dims: [part(=hd within head pair), qk, i(double-row), s]
            # QK_DR: both DR planes carry the same data (score doubles;
            # compensated in the exp scale) -> no zero-plane memset needed.
            # Legacy path: i=1 plane zeroed once on DVE.
            qk8 = [cpool.tile([128, 2, 2, S], F8E4, name=f"qk8_{b_}")
                   for b_ in range(2)]
            if not QK_DR:
                for b_ in range(2):
                    nc.vector.memset(qk8[b_][:], 0.0)

            # effective exp scale: scores are doubled by the replicated DR
            # planes when QK_DR is on
            esc = SCALE * (0.5 if QK_DR else 1.0)
            alpha = 1477.3193223344908 * esc

            # exp dispatch: round-robin over ACT/DVE/GPSIMD per EXP_PATTERN
            exp_ctr = [0]

            def emit_exp(dst, src):
                kind = EXP_PATTERN[exp_ctr[0] % len(EXP_PATTERN)]
                exp_ctr[0] += 1
                if skip_exp:
                    nc.vector.tensor_copy(dst, src)
                elif kind == 'A':
                    nc.scalar.activation(dst, src, Exp, scale=esc)
                elif kind == 'D':
                    nc.vector.tensor_scalar(dst.bitcast(I16), src,
                                            alpha, BETA, op0=MULT, op1=ADD)
                else:
                    nc.gpsimd.tensor_scalar(dst.bitcast(I16), src,
                                            alpha, BETA, op0=MULT, op1=ADD)

            # ctx^T accumulator [d-part packed by pair, pair, q] fp16
            ctxT = cpool.tile([128, 4, S], F16)

            # ---- stage1 helpers ----
            def s1_qk_half(p, j, n, half, ps):
                """half of the contraction for pair p, j=0 q/1 k, chunk n."""
                foff = 128 * p + 512 * j
                for kc in range(4 * half, 4 * half + 4):
                    if QK_DR:
                        nc.tensor.matmul(
                            ps[:], wqk8[:, :, kc, foff:foff + 128],
                            x8[:, :, kc, 512 * n:512 * (n + 1)],
                            start=(kc == 0), stop=(kc == 7), perf_mode=DR)
                    else:
                        nc.tensor.matmul(
                            ps[:], wqk[:, kc, foff:foff + 128],
                            x16[:, kc, 512 * n:512 * (n + 1)],
                            start=(kc == 0), stop=(kc == 7))
                if half == 1:
                    if QK_DR:
                        for pl in range(2):
                            nc.gpsimd.tensor_scalar(
                                qk8[p % 2][:, j, pl, 512 * n:512 * (n + 1)],
                                ps[:], 1.0 / WSCL,
                                bqk[:, 4 * j + p:4 * j + p + 1],
                                op0=MULT, op1=ADD)
                    else:
                        nc.gpsimd.tensor_scalar_add(
                            qk8[p % 2][:, j, 0, 512 * n:512 * (n + 1)], ps[:],
                            bqk[:, 4 * j + p:4 * j + p + 1])

            def s1_qk_tile(p, j, n):
                ps = sh_ps.tile([128, 512], F32, name="s1", tag="sh")
                s1_qk_half(p, j, n, 0, ps)
                s1_qk_half(p, j, n, 1, ps)

            def s1_qk_items(p, j, n):
                ps = sh_ps.tile([128, 512], F32, name="s1", tag="sh")
                return [(lambda: s1_qk_half(p, j, n, 0, ps)),
                        (lambda: s1_qk_half(p, j, n, 1, ps))]

            def s1_v_tile(p, t):
                """v for pair p (128 feats), seq tile t (128 rows)."""
                ps = sh_ps.tile([128, 128], F32, name="s1v", tag="sh")
                for kc in range(8):
                    nc.tensor.matmul(
                        ps[:], x16[:, kc, 128 * t:128 * (t + 1)],
                        wv[:, kc, 128 * p:128 * (p + 1)],
                        start=(kc == 0), stop=(kc == 7))
                nc.gpsimd.tensor_copy(
                    v_sb[:, t, 2 * p:2 * p + 2, 0:HD],
                    ps.rearrange("a (h d) -> a h d", h=2))

            def s1_v_tile_dr(t):
                """v for ALL 8 heads (512 feats), seq tile t, fp8 DoubleRow."""
                ps = sh_ps.tile([128, 512], F32, name="s1v", tag="sh")
                for kc in range(8):
                    nc.tensor.matmul(
                        ps[:], x8[:, :, kc, 128 * t:128 * (t + 1)],
                        wv8[:, :, kc, :],
                        start=(kc == 0), stop=(kc == 7), perf_mode=DR)
                nc.gpsimd.tensor_scalar(
                    v_sb[:, t, :, 0:HD],
                    ps.rearrange("a (h d) -> a h d", h=8),
                    1.0 / WSCL, None, op0=MULT)

            # ---- deferred PE work queue: popped between score/exp pairs so
            # the ACT engine (bottleneck) never starves while PE does the
            # ctx/transpose/stage1/outproj work of earlier units ----
            from collections import deque
            work_q = deque()

            def pop_work(k):
                for _ in range(k):
                    if work_q:
                        work_q.popleft()()

            def mk_chain(p, qc, hi, qt, lhsT_fn, cn):
                def run():
                    h = 2 * p + hi
                    cps = ctx_ps.tile([128, HD + 1], F32,
                                      name="ctx", tag="ctx")
                    for sk in range(NSK):
                        nc.tensor.matmul(
                            cps[:],
                            lhsT_fn(hi, qt, sk),
                            v_sb[:, sk, h, :],
                            start=(sk == 0), stop=(sk == NSK - 1))
                    rcp = rcp_pool.tile([128, 1], F32, name="rc",
                                        tag="rc")
                    nc.vector.reciprocal_approx_fast(
                        rcp[:], cps[:, HD:HD + 1])
                    nc.vector.scalar_tensor_tensor(
                        cn[:, hi, :], cps[:, 0:HD], rcp[:], bv[:, h, :],
                        op0=MULT, op1=ADD)
                return run

            def mk_transp(p, qc, qt, cn):
                def run():
                    tp = ctx_ps.tile([128, 128], F16, name="tp", tag="ctx")
                    nc.tensor.matmul(
                        tp[0:64, :], cn[:, 0, :], ident[:],
                        start=True, stop=True, is_transpose=True,
                        tile_position=(0, 0))
                    nc.tensor.matmul(
                        tp[64:128, :], cn[:, 1, :], ident[:],
                        start=True, stop=True, is_transpose=True,
                        tile_position=(0, 64))
                    qoff = 512 * qc + 128 * qt
                    nc.vector.tensor_copy(
                        ctxT[:, p, qoff:qoff + 128], tp[:])
                return run

            def mk_oproj(qc, qt, dc):
                def run():
                    qoff = 512 * qc + 128 * qt
                    ops = sc_ps.tile([128, 512], F32, name="op", tag="sc")
                    for c in range(4):
                        nc.tensor.matmul(
                            ops[:], ctxT[:, c, qoff:qoff + 128],
                            wout[:, c, 512 * dc:512 * (dc + 1)],
                            start=(c == 0), stop=(c == 3))
                    o16 = out_pool.tile([128, 512], F16, name="o")
                    nc.vector.tensor_copy(o16[:], ops[:])
                    nc.sync.dma_start(
                        out_d[qoff:qoff + 128,
                              512 * dc:512 * (dc + 1)], o16[:])
                return run

            def push_tail_hi(p, qc, hi, lhsT_fn, cns):
                """Queue head hi's ctx chains; after hi=1 also transposes
                and (for the last pair) this q-chunk's out-projection.
                Transposes are staggered one qt behind the hi=1 chains so
                the DVE rcp+stt latency is hidden by the next chain's
                matmuls instead of stalling PE."""
                if skip_tail:
                    return
                if hi == 0:
                    for qt in range(4):
                        work_q.append(mk_chain(p, qc, 0, qt, lhsT_fn,
                                               cns[qt]))
                    return
                order = [("c", 0), ("c", 1), ("t", 0), ("c", 2), ("t", 1),
                         ("c", 3), ("t", 2), ("t", 3)]
                for kind, qt in order:
                    if kind == "c":
                        work_q.append(mk_chain(p, qc, 1, qt, lhsT_fn,
                                               cns[qt]))
                    else:
                        work_q.append(mk_transp(p, qc, qt, cns[qt]))
                if p == 3:
                    for qt in range(4):
                        work_q.append(mk_oproj(qc, qt, 0))
                        work_q.append(mk_oproj(qc, qt, 1))

            # ---- inline stage1 schedule: stage1 matmuls/quants are emitted
            # at fixed score-slots inside each unit (deadline-driven), NOT
            # via the work queue -- the queue holds only tail work (chains/
            # transposes/oproj) with a guaranteed ~1-unit lag. ----
            def s1_tile_halves(p, j, n):
                holder = []

                def h0():
                    ps = sh_ps.tile([128, 512], F32, name="s1", tag="sh")
                    holder.append(ps)
                    s1_qk_half(p, j, n, 0, ps)

                def h1():
                    s1_qk_half(p, j, n, 1, holder[0])
                return h0, h1

            inline_work = {u_: {} for u_ in range(16)}

            def add_inline(u_, sl, fn):
                inline_work[u_].setdefault(sl, []).append(fn)

            def add_tile(u_, sl0, sl1, p, j, n):
                h0, h1 = s1_tile_halves(p, j, n)
                add_inline(u_, sl0, h0)
                add_inline(u_, sl1, h1)

            # pair 0 remaining q/k tiles (k chunks feed this unit's own
            # scores -- earliest slots), v-proj for all pairs in unit 0
            add_tile(0, 0, 1, 0, 1, 1)
            add_tile(0, 2, 3, 0, 1, 2)
            add_tile(0, 4, 5, 0, 1, 3)
            add_tile(0, 6, 7, 0, 0, 1)
            add_tile(1, 0, 1, 0, 0, 2)
            add_tile(1, 2, 3, 0, 0, 3)
            if not skip_v:
                for t in range(NSK):
                    add_inline(0, 8 + t, (lambda t=t: s1_v_tile_dr(t))
                               if V_DR else (lambda t=t: s1_v_tile(0, t)))
            # pair p+1 tiles spread over pair p's units
            for p_ in range(3):
                for qc_ in range(4):
                    jn = [(1, 0), (1, 1)] if qc_ == 0 else \
                         [(1, 2), (1, 3)] if qc_ == 1 else \
                         [(0, 0), (0, 1)] if qc_ == 2 else \
                         [(0, 2), (0, 3)]
                    u_ = 4 * p_ + qc_
                    base = 6
                    step = 8
                    if p_ == 0:
                        base = 24 if qc_ == 0 else 8
                        if qc_ == 0:
                            step = 2
                    for (j, n) in jn:
                        add_tile(u_, base, base + 1, p_ + 1, j, n)
                        base += step

            # ---- prologue: k chunk 0 + q chunk 0 of pair 0 inline ----
            s1_qk_tile(0, 1, 0)
            s1_qk_tile(0, 0, 0)

            # ---- main software-pipelined unit loop ----
            for p in range(4):
                for qc in range(4):
                    u = 4 * p + qc
                    last_unit = (u == 15)
                    buf = qk8[p % 2]
                    cns = [ctxN_pool.tile([128, 2, HD], F16, name="cnq",
                                          tag="cn") for _ in range(4)]
                    slot = 0
                    if last_unit:
                        # qt-granular mini-units: the tail of each q-tile
                        # cascades behind its own exps, shrinking the drain
                        for qt in range(4):
                            qoff = 512 * qc + 128 * qt
                            minis = {}
                            for hi in range(2):
                                tiles = []
                                for quarter in range(4):
                                    scp = sc_ps.tile([128, 4, 128], F32,
                                                     name="scm", tag="sc")
                                    for s4 in range(4):
                                        sk = 4 * quarter + s4
                                        nc.tensor.matmul(
                                            scp[:, s4, :],
                                            buf[64 * hi:64 * (hi + 1), 1, :,
                                                128 * sk:128 * (sk + 1)],
                                            buf[64 * hi:64 * (hi + 1), 0, :,
                                                qoff:qoff + 128],
                                            start=True, stop=True,
                                            perf_mode=DR)
                                    et = expT_pool.tile([128, 4, 128], F16,
                                                        name="em", tag="expT")
                                    emit_exp(et[:], scp[:])
                                    tiles.append(et)
                                    pop_work(1)
                                minis[hi] = tiles

                            def mini_fn(minis):
                                def f(hi, qt_, sk):
                                    return minis[hi][sk // 4][:, sk % 4, :]
                                return f
                            lf = mini_fn(minis)
                            work_q.append(mk_chain(p, qc, 0, qt, lf,
                                                    cns[qt]))
                            work_q.append(mk_chain(p, qc, 1, qt, lf,
                                                   cns[qt]))
                            work_q.append(mk_transp(p, qc, qt, cns[qt]))
                            work_q.append(mk_oproj(qc, qt, 0))
                            work_q.append(mk_oproj(qc, qt, 1))
                        continue
                    expTs = {}
                    for hi in range(2):
                        expTs[hi] = expT_pool.tile([128, NSK, 512], F16,
                                                   name=f"e{hi}", tag="expT")
                        for sk in range(NSK):
                            scp = sc_ps.tile([128, 512], F32, name="sc",
                                             tag="sc")
                            nc.tensor.matmul(
                                scp[:],
                                buf[64 * hi:64 * (hi + 1), 1, :,
                                    128 * sk:128 * (sk + 1)],
                                buf[64 * hi:64 * (hi + 1), 0, :,
                                    512 * qc:512 * (qc + 1)],
                                start=True, stop=True, perf_mode=DR)
                            emit_exp(expTs[hi][:, sk, :], scp[:])
                            for fn in inline_work[u].get(slot, []):
                                fn()
                            if u > 0 and slot % 8 in (2, 4, 7):
                                pop_work(1)
                            if p == 3 and slot % 16 == 9:
                                pop_work(1)
                            slot += 1
                        if debug and u == 0 and hi == 0:
                            nc.sync.dma_start(dbg_expT[:], expTs[0][:])
                        def exp_fn(expTs, hi):
                            def f(hi_, qt, sk):
                                return expTs[hi_][:, sk,
                                                  128 * qt:128 * (qt + 1)]
                            return f
                        push_tail_hi(p, qc, hi, exp_fn(expTs, hi), cns)

            # drain
            while work_q:
                work_q.popleft()()
            if debug:
                nc.sync.dma_start(dbg_ctxT[:], ctxT[:])
                nc.sync.dma_start(dbg_v[:], v_sb[:])
                nc.sync.dma_start(dbg_qk8[:], qk8[0][:])

    nc.compile()
    return nc


# ---------------------------------------------------------------------------
# host side: shard, run SPMD, gather
# ---------------------------------------------------------------------------

_RUNNER = None


def _make_runner(nc, n_cores):
    """Jit-once SPMD runner via PJRT (axon)."""
    import jax
    from jax.sharding import Mesh, PartitionSpec
    from jax.experimental.shard_map import shard_map
    from concourse import bass2jax
    from concourse.bass2jax import _bass_exec_p, install_neuronx_cc_hook

    install_neuronx_cc_hook()
    partition_name = nc.partition_id_tensor.name if nc.partition_id_tensor else None

    in_names, out_names, out_avals, zero_outs = [], [], [], []
    for alloc in nc.m.functions[0].allocations:
        if not isinstance(alloc, mybir.MemoryLocationSet):
            continue
        name = alloc.memorylocations[0].name
        if alloc.kind == "ExternalInput":
            if name != partition_name:
                in_names.append(name)
        elif alloc.kind == "ExternalOutput":
            out_names.append(name)
            shape = tuple(alloc.tensor_shape)
            dtype = mybir.dt.np(alloc.dtype)
            out_avals.append(jax.core.ShapedArray(shape, dtype))
            zero_outs.append(np.zeros(shape, dtype))
    n_params = len(in_names)
    n_outs = len(out_avals)
    all_in_names = list(in_names) + list(out_names)
    if partition_name is not None:
        all_in_names.append(partition_name)

    def _body(*args):
        operands = list(args)
        if partition_name is not None:
            operands.append(bass2jax.partition_id_tensor())
        outs = _bass_exec_p.bind(
            *operands,
            out_avals=tuple(out_avals),
            in_names=tuple(all_in_names),
            out_names=tuple(out_names),
            lowering_input_output_aliases=(),
            sim_require_finite=True,
            sim_require_nnan=True,
            nc=nc,
        )
        return tuple(outs)

    devices = jax.devices()[:n_cores]
    mesh = Mesh(np.asarray(devices), ("core",))
    in_specs = (PartitionSpec("core"),) * (n_params + n_outs)
    out_specs = (PartitionSpec("core"),) * n_outs
    jitted = jax.jit(
        shard_map(_body, mesh=mesh, in_specs=in_specs, out_specs=out_specs,
                  check_rep=False),
        keep_unused=True,
    )

    def run(in_maps):
        concat_in = [
            np.concatenate([np.asarray(in_maps[c][n]) for c in range(n_cores)],
                           axis=0)
            for n in in_names
        ]
        concat_zero = [
            np.zeros((n_cores * z.shape[0], *z.shape[1:]), z.dtype)
            for z in zero_outs
        ]
        out_arrs = jitted(*concat_in, *concat_zero)
        jax.block_until_ready(out_arrs)
        return [
            {n: np.asarray(out_arrs[i]).reshape(n_cores, *out_avals[i].shape)[c]
             for i, n in enumerate(out_names)}
            for c in range(n_cores)
        ]

    return run


def _shard_inputs(qkv, W_in, b_in, W_out, b_out):
    """Build the 8 per-core input dicts."""
    f16 = np.float16
    x = np.asarray(qkv, np.float32)
    W_in = np.asarray(W_in, np.float32)
    b_in = np.asarray(b_in, np.float32)
    W_out = np.asarray(W_out, np.float32)
    ident = np.eye(128, dtype=f16)

    f8 = mybir.dt.np(mybir.dt.float8e4)
    in_maps = []
    for c in range(N_CORES):
        b, g = divmod(c, 2)
        qs = slice(512 * g, 512 * (g + 1))
        ks = slice(1024 + 512 * g, 1024 + 512 * (g + 1))
        vs = slice(2048 + 512 * g, 2048 + 512 * (g + 1))
        xT = np.ascontiguousarray(x[b].T)                     # [D, S]
        # wout[p, c_, dout] = W_out[512*g + 128*c_ + p, dout]
        wout = W_out[512 * g:512 * (g + 1), :].reshape(4, 128, D) \
            .transpose(1, 0, 2).astype(f16)
        bqk = np.concatenate([b_in[qs], b_in[ks]]).reshape(8, 128).T \
            .astype(np.float32)
        bqk = np.ascontiguousarray(bqk)
        bv = np.broadcast_to(b_in[vs].reshape(8, HD), (128, 8, HD)) \
            .astype(f16)
        in_map = {
            "wout": wout,
            "bqk": bqk,
            "bv": np.ascontiguousarray(bv),
            "ident": ident,
        }
        if QK_DR or V_DR:
            # x8[p, pl, kc, s] = xT[128*kc + 64*pl + p, s]
            in_map["x8"] = np.ascontiguousarray(
                xT.reshape(8, 2, 64, S).transpose(2, 1, 0, 3).astype(f8))
        if not (QK_DR and V_DR):
            # x16[p, kc, s] = xT[128*kc+p, s]
            in_map["x16"] = xT.reshape(8, 128, S).transpose(1, 0, 2) \
                .astype(f16)
        if QK_DR:
            wqk_full = np.concatenate([W_in[:, qs], W_in[:, ks]],
                                      axis=1) * WSCL            # [D, 1024]
            in_map["wqk8"] = np.ascontiguousarray(
                wqk_full.reshape(8, 2, 64, 1024).transpose(2, 1, 0, 3)
                .astype(f8))
        else:
            # wqk[p, kc, f]: f 0..511 q feats, 512..1023 k feats
            wq = W_in[:, qs].reshape(8, 128, 512).transpose(1, 0, 2)
            wk = W_in[:, ks].reshape(8, 128, 512).transpose(1, 0, 2)
            in_map["wqk"] = np.concatenate([wq, wk], axis=2).astype(f16)
        if V_DR:
            in_map["wv8"] = np.ascontiguousarray(
                (W_in[:, vs] * WSCL).reshape(8, 2, 64, 512)
                .transpose(2, 1, 0, 3).astype(f8))
        else:
            in_map["wv"] = W_in[:, vs].reshape(8, 128, 512) \
                .transpose(1, 0, 2).astype(f16)
        in_maps.append(in_map)
    return in_maps


def kernel(qkv, W_in, b_in, W_out, b_out):
    global _RUNNER
    if _RUNNER is None:
        nc = build_nc()
        _RUNNER = _make_runner(nc, N_CORES)
    in_maps = _shard_inputs(qkv, W_in, b_in, W_out, b_out)
    results = _RUNNER(in_maps)
    b_out = np.asarray(b_out, np.float32)
    out = np.empty((B, S, D), np.float32)
    for b in range(B):
        out[b] = (results[2 * b]["out"].astype(np.float32)
                  + results[2 * b + 1]["out"].astype(np.float32) + b_out)
    return out


if __name__ == "__main__":
    rng = np.random.default_rng(0)
    qkv = rng.standard_normal((B, S, D)).astype(np.float32)
    sc = 1.0 / np.sqrt(D)
    W_in = rng.uniform(-sc, sc, (D, 3 * D)).astype(np.float32)
    b_in = rng.uniform(-sc, sc, (3 * D,)).astype(np.float32)
    W_out = rng.uniform(-sc, sc, (D, D)).astype(np.float32)
    b_out = rng.uniform(-sc, sc, (D,)).astype(np.float32)
    got = kernel(qkv, W_in, b_in, W_out, b_out)
    print("kernel ran, output shape", got.shape)



# revision 33
# speedup vs baseline: 1.0039x; 1.0039x over previous
"""Multi-head self-attention on 8 TRN2 NeuronCores.

Sharding: core c -> (batch b = c//2, head-half g = c%2, i.e. 8 of 16 heads).
Each core computes qkv-proj + attention + out-proj partial for its 8 heads;
host sums the two partials per batch and adds b_out.

Design (v2):
- stage1 q,k,v projections in fp16 (1 cyc/row), q/k results quantized to
  fp8e4 on the PSUM->SBUF copy (bias added via per-partition tensor_scalar).
- scores as zero-padded DoubleRow fp8 matmuls (0.5 cyc/row): operands
  [64, 2, *] with the i=1 plane zeroed; out tile [128 k-pos, 512 q-pos].
- exp on ACT (scale=0.125 applied in the activation), fp16 out.
- ctx computed transposed: out [128 q, 65] with lhsT = exp tile (stationary)
  and rhs = V' [128 k, 65] whose 65th column is ones -> denominator lands in
  out[:, 64] = per-partition scalar. Normalization + V-bias is then a single
  scalar_tensor_tensor (mult, add) per (head, q-tile).
- ctx^T via PE transpose (fp16, identity rhs) packing head pairs into
  [128, 128] PSUM tiles; out-projection over the packed [d, q] layout,
  fp16 output DMA'd per tile; host sums core pairs + b_out.
- software pipelining: unit (pair, qc) emits its 32 score matmuls + 16 exps,
  then the previous unit's ctx/norm/transpose tail, then next-pair stage1
  or out-projection work, keeping ACT (the bottleneck) saturated.
"""
import sys
sys.path.insert(0, '/opt/trn_rl_repo')

import numpy as np

import concourse.bass as bass
import concourse.mybir as mybir
import concourse.tile as tile
from concourse import bacc

F32 = mybir.dt.float32
F16 = mybir.dt.float16
F8E4 = mybir.dt.float8e4
I16 = mybir.dt.int16
DR = mybir.MatmulPerfMode.DoubleRow
Exp = mybir.ActivationFunctionType.Exp
MULT = mybir.AluOpType.mult
ADD = mybir.AluOpType.add

B, S, D = 4, 2048, 1024
H, HD = 16, 64
N_CORES = 8
NSK = S // 128            # 16 k-chunks of 128
NQT = S // 128            # 16 q-tiles of 128
SCALE = 0.125             # 1/sqrt(HD)

# Schraudolph exp for DVE/GPSIMD offload: fp16 bits = trunc(ALPHA*s + BETA)
# approximates exp(s * SCALE) to ~3% max rel error (error cancels partially
# in the softmax ratio). ALPHA = 1024*log2(e)*SCALE.
ALPHA = 184.6649652337873
BETA = 15316.431477991726

# per-unit exp engine assignment (16 slots): A=ACT native exp,
# D=DVE Schraudolph, P=GPSIMD Schraudolph
# 32 slots, 16A/9D/7P: balances ACT/DVE/GPSIMD busy at ~155-165us each
EXP_PATTERN = "ADPAADPAADPAADAPADPAADPAADPAADAD"

# stage1 projections as fp8 DoubleRow matmuls (halves PE cost of each).
# Weights are pre-scaled by WSCL on the host so W_in/W_out values
# (~U(-1/32,1/32)) sit mid-range in fp8e4m3 instead of subnormal;
# compensated by 1/WSCL on the PSUM->SBUF copies.
QK_DR = True
V_DR = True
WSCL = 64.0


def build_nc(skip_tail=False, skip_exp=False, sc_bufs=4, skip_v=False, fake_in=False, debug=False):
    nc = bacc.Bacc(None, target_bir_lowering=False)

    if QK_DR or V_DR:
        x8_d = nc.dram_tensor("x8", [64, 2, 8, S], F8E4, kind="ExternalInput")
    if not (QK_DR and V_DR):
        x16_d = nc.dram_tensor("x16", [128, 8, S], F16, kind="ExternalInput")
    if QK_DR:
        wqk8_d = nc.dram_tensor("wqk8", [64, 2, 8, 1024], F8E4,
                                kind="ExternalInput")
    else:
        wqk_d = nc.dram_tensor("wqk", [128, 8, 1024], F16,
                               kind="ExternalInput")
    if V_DR:
        wv8_d = nc.dram_tensor("wv8", [64, 2, 8, 512], F8E4,
                               kind="ExternalInput")
    else:
        wv_d = nc.dram_tensor("wv", [128, 8, 512], F16, kind="ExternalInput")
    wout_d = nc.dram_tensor("wout", [128, 4, D], F16, kind="ExternalInput")
    bqk_d = nc.dram_tensor("bqk", [128, 8], F32, kind="ExternalInput")
    bv_d = nc.dram_tensor("bv", [128, 8, HD], F16, kind="ExternalInput")
    ident_d = nc.dram_tensor("ident", [128, 128], F16, kind="ExternalInput")
    out_d = nc.dram_tensor("out", [S, D], F16, kind="ExternalOutput")
    if debug:
        dbg_qk8 = nc.dram_tensor("dbg_qk8", [128, 2, 2, S], F8E4,
                                 kind="ExternalOutput")
        dbg_expT = nc.dram_tensor("dbg_expT", [128, NSK, 512], F16,
                                  kind="ExternalOutput")
        dbg_ctxT = nc.dram_tensor("dbg_ctxT", [128, 4, S], F16,
                                  kind="ExternalOutput")
        dbg_v = nc.dram_tensor("dbg_v", [128, NSK, 8, HD + 1], F16,
                               kind="ExternalOutput")

    with tile.TileContext(nc) as tc:
        with (
            tc.tile_pool(name="const", bufs=1) as cpool,
            tc.tile_pool(name="expT", bufs=4) as expT_pool,
            tc.tile_pool(name="ctxN", bufs=4) as ctxN_pool,
            tc.tile_pool(name="rcp", bufs=4) as rcp_pool,
            tc.tile_pool(name="osb", bufs=2) as out_pool,
            tc.tile_pool(name="scps", bufs=sc_bufs, space="PSUM") as sc_ps,
            tc.tile_pool(name="ctxps", bufs=2, space="PSUM") as ctx_ps,
            tc.tile_pool(name="shps", bufs=2, space="PSUM") as sh_ps,
        ):
            # ---- constants / persistent tiles (DMAs ordered so the
            # prologue's pair-0 stage1 work can start immediately) ----
            if QK_DR or V_DR:
                x8 = cpool.tile([64, 2, 8, S], F8E4)
            if not (QK_DR and V_DR):
                x16 = cpool.tile([128, 8, S], F16)
            if QK_DR:
                wqk8 = cpool.tile([64, 2, 8, 1024], F8E4)
            else:
                wqk = cpool.tile([128, 8, 1024], F16)
            bqk = cpool.tile([128, 8], F32)
            if V_DR:
                wv8 = cpool.tile([64, 2, 8, 512], F8E4)
            else:
                wv = cpool.tile([128, 8, 512], F16)
            wout = cpool.tile([128, 4, D], F16)
            bv = cpool.tile([128, 8, HD], F16)
            ident = cpool.tile([128, 128], F16)
            # critical-path DMAs on SP, halves sized to unblock the first
            # stage1 half-tile ASAP; bulk loads ride the ACT hwdge queue
            nc.sync.dma_start(bqk[:], bqk_d[:])
            if QK_DR:
                nc.sync.dma_start(wqk8[:, :, :, 512:640], wqk8_d[:, :, :, 512:640])
            else:
                nc.sync.dma_start(wqk[:, :, 512:640], wqk_d[:, :, 512:640])
            if QK_DR or V_DR:
                nc.sync.dma_start(x8[:, :, :, 0:512], x8_d[:, :, :, 0:512])
            if not (QK_DR and V_DR):
                nc.sync.dma_start(x16[:, :, 0:512], x16_d[:, :, 0:512])
            if QK_DR:
                nc.sync.dma_start(wqk8[:, :, :, 0:128], wqk8_d[:, :, :, 0:128])
            else:
                nc.sync.dma_start(wqk[:, :, 0:128], wqk_d[:, :, 0:128])
            for n in range(1, 4):
                if QK_DR or V_DR:
                    nc.sync.dma_start(x8[:, :, :, 512 * n:512 * (n + 1)],
                                      x8_d[:, :, :, 512 * n:512 * (n + 1)])
                if not (QK_DR and V_DR):
                    nc.sync.dma_start(x16[:, :, 512 * n:512 * (n + 1)],
                                      x16_d[:, :, 512 * n:512 * (n + 1)])
            if V_DR:
                nc.sync.dma_start(wv8[:], wv8_d[:])
            else:
                nc.sync.dma_start(wv[:], wv_d[:])
            nc.sync.dma_start(bv[:], bv_d[:])
            nc.sync.dma_start(ident[:], ident_d[:])
            for p_ in range(1, 4):
                if QK_DR:
                    nc.sync.dma_start(
                        wqk8[:, :, :, 512 + 128 * p_:640 + 128 * p_],
                        wqk8_d[:, :, :, 512 + 128 * p_:640 + 128 * p_])
                    nc.sync.dma_start(
                        wqk8[:, :, :, 128 * p_:128 * (p_ + 1)],
                        wqk8_d[:, :, :, 128 * p_:128 * (p_ + 1)])
                else:
                    nc.sync.dma_start(
                        wqk[:, :, 512 + 128 * p_:640 + 128 * p_],
                        wqk_d[:, :, 512 + 128 * p_:640 + 128 * p_])
                    nc.sync.dma_start(wqk[:, :, 128 * p_:128 * (p_ + 1)],
                                      wqk_d[:, :, 128 * p_:128 * (p_ + 1)])
            nc.sync.dma_start(wout[:], wout_d[:])

            # PE p-state warm-up: the cost model charges cold-clock cycles
            # until the PE has been continuously busy for 3us at prep time.
            # Run a dependency-free accumulating matmul chain from t=0 so the
            # real prologue matmuls are prepped at the full 2.4 GHz clock.
            wrm = cpool.tile([128, 512], F16)
            nc.vector.memset(wrm[:], 0.125)
            # preload the ACT exp table at t~0 (1.3us) so the first real
            # exp doesn't eat the table-load latency mid-pipeline
            actwarm = cpool.tile([128, 1], F16)
            nc.scalar.activation(actwarm[:], wrm[:, 0:1], Exp, scale=1.0)
            wps = sh_ps.tile([128, 512], F32, name="wps", tag="sh")
            NWARM = 14
            for i in range(NWARM):
                nc.tensor.matmul(wps[:], wrm[:, 0:128], wrm[:],
                                 start=(i == 0), stop=(i == NWARM - 1))

            # V': [s%128, sk, head, hd+1]; [..,64] = 1.0 for denominators
            v_sb = cpool.tile([128, NSK, 8, HD + 1], F16)
            nc.vector.memset(v_sb[:, :, :, HD], 1.0)

            # q/k fp8 double-buffered (ping-pong by pair parity).
            # dims: [part(=hd within head pair), qk, i(double-row), s]
            # QK_DR: both DR planes carry the same data (score doubles;
            # compensated in the exp scale) -> no zero-plane memset needed.
            # Legacy path: i=1 plane zeroed once on DVE.
            qk8 = [cpool.tile([128, 2, 2, S], F8E4, name=f"qk8_{b_}")
                   for b_ in range(2)]
            if not QK_DR:
                for b_ in range(2):
                    nc.vector.memset(qk8[b_][:], 0.0)

            # effective exp scale: scores are doubled by the replicated DR
            # planes when QK_DR is on
            esc = SCALE * (0.5 if QK_DR else 1.0)
            alpha = 1477.3193223344908 * esc

            # exp dispatch: round-robin over ACT/DVE/GPSIMD per EXP_PATTERN
            exp_ctr = [0]

            def emit_exp(dst, src):
                kind = EXP_PATTERN[exp_ctr[0] % len(EXP_PATTERN)]
                exp_ctr[0] += 1
                if skip_exp:
                    nc.vector.tensor_copy(dst, src)
                elif kind == 'A':
                    nc.scalar.activation(dst, src, Exp, scale=esc)
                elif kind == 'D':
                    nc.vector.tensor_scalar(dst.bitcast(I16), src,
                                            alpha, BETA, op0=MULT, op1=ADD)
                else:
                    nc.gpsimd.tensor_scalar(dst.bitcast(I16), src,
                                            alpha, BETA, op0=MULT, op1=ADD)

            # ctx^T accumulator [d-part packed by pair, pair, q] fp16
            ctxT = cpool.tile([128, 4, S], F16)

            # ---- stage1 helpers ----
            def s1_qk_half(p, j, n, half, ps):
                """half of the contraction for pair p, j=0 q/1 k, chunk n."""
                foff = 128 * p + 512 * j
                for kc in range(4 * half, 4 * half + 4):
                    if QK_DR:
                        nc.tensor.matmul(
                            ps[:], wqk8[:, :, kc, foff:foff + 128],
                            x8[:, :, kc, 512 * n:512 * (n + 1)],
                            start=(kc == 0), stop=(kc == 7), perf_mode=DR)
                    else:
                        nc.tensor.matmul(
                            ps[:], wqk[:, kc, foff:foff + 128],
                            x16[:, kc, 512 * n:512 * (n + 1)],
                            start=(kc == 0), stop=(kc == 7))
                if half == 1:
                    if QK_DR:
                        for pl in range(2):
                            nc.gpsimd.tensor_scalar(
                                qk8[p % 2][:, j, pl, 512 * n:512 * (n + 1)],
                                ps[:], 1.0 / WSCL,
                                bqk[:, 4 * j + p:4 * j + p + 1],
                                op0=MULT, op1=ADD)
                    else:
                        nc.gpsimd.tensor_scalar_add(
                            qk8[p % 2][:, j, 0, 512 * n:512 * (n + 1)], ps[:],
                            bqk[:, 4 * j + p:4 * j + p + 1])

            def s1_qk_tile(p, j, n):
                ps = sh_ps.tile([128, 512], F32, name="s1", tag="sh")
                s1_qk_half(p, j, n, 0, ps)
                s1_qk_half(p, j, n, 1, ps)

            def s1_qk_items(p, j, n):
                ps = sh_ps.tile([128, 512], F32, name="s1", tag="sh")
                return [(lambda: s1_qk_half(p, j, n, 0, ps)),
                        (lambda: s1_qk_half(p, j, n, 1, ps))]

            def s1_v_tile(p, t):
                """v for pair p (128 feats), seq tile t (128 rows)."""
                ps = sh_ps.tile([128, 128], F32, name="s1v", tag="sh")
                for kc in range(8):
                    nc.tensor.matmul(
                        ps[:], x16[:, kc, 128 * t:128 * (t + 1)],
                        wv[:, kc, 128 * p:128 * (p + 1)],
                        start=(kc == 0), stop=(kc == 7))
                nc.gpsimd.tensor_copy(
                    v_sb[:, t, 2 * p:2 * p + 2, 0:HD],
                    ps.rearrange("a (h d) -> a h d", h=2))

            def s1_v_tile_dr(t):
                """v for ALL 8 heads (512 feats), seq tile t, fp8 DoubleRow."""
                ps = sh_ps.tile([128, 512], F32, name="s1v", tag="sh")
                for kc in range(8):
                    nc.tensor.matmul(
                        ps[:], x8[:, :, kc, 128 * t:128 * (t + 1)],
                        wv8[:, :, kc, :],
                        start=(kc == 0), stop=(kc == 7), perf_mode=DR)
                nc.gpsimd.tensor_scalar(
                    v_sb[:, t, :, 0:HD],
                    ps.rearrange("a (h d) -> a h d", h=8),
                    1.0 / WSCL, None, op0=MULT)

            # ---- deferred PE work queue: popped between score/exp pairs so
            # the ACT engine (bottleneck) never starves while PE does the
            # ctx/transpose/stage1/outproj work of earlier units ----
            from collections import deque
            work_q = deque()

            def pop_work(k):
                for _ in range(k):
                    if work_q:
                        work_q.popleft()()

            def mk_chain(p, qc, hi, qt, lhsT_fn, cn):
                def run():
                    h = 2 * p + hi
                    cps = ctx_ps.tile([128, HD + 1], F32,
                                      name="ctx", tag="ctx")
                    for sk in range(NSK):
                        nc.tensor.matmul(
                            cps[:],
                            lhsT_fn(hi, qt, sk),
                            v_sb[:, sk, h, :],
                            start=(sk == 0), stop=(sk == NSK - 1))
                    rcp = rcp_pool.tile([128, 1], F32, name="rc",
                                        tag="rc")
                    nc.vector.reciprocal_approx_fast(
                        rcp[:], cps[:, HD:HD + 1])
                    nc.vector.scalar_tensor_tensor(
                        cn[:, hi, :], cps[:, 0:HD], rcp[:], bv[:, h, :],
                        op0=MULT, op1=ADD)
                return run

            def mk_transp(p, qc, qt, cn):
                def run():
                    tp = sh_ps.tile([128, 128], F16, name="tp", tag="sh")
                    nc.tensor.matmul(
                        tp[0:64, :], cn[:, 0, :], ident[:],
                        start=True, stop=True, is_transpose=True,
                        tile_position=(0, 0))
                    nc.tensor.matmul(
                        tp[64:128, :], cn[:, 1, :], ident[:],
                        start=True, stop=True, is_transpose=True,
                        tile_position=(0, 64))
                    qoff = 512 * qc + 128 * qt
                    nc.vector.tensor_copy(
                        ctxT[:, p, qoff:qoff + 128], tp[:])
                return run

            def mk_oproj(qc, qt, dc):
                def run():
                    qoff = 512 * qc + 128 * qt
                    ops = sc_ps.tile([128, 512], F32, name="op", tag="sc")
                    for c in range(4):
                        nc.tensor.matmul(
                            ops[:], ctxT[:, c, qoff:qoff + 128],
                            wout[:, c, 512 * dc:512 * (dc + 1)],
                            start=(c == 0), stop=(c == 3))
                    o16 = out_pool.tile([128, 512], F16, name="o")
                    nc.vector.tensor_copy(o16[:], ops[:])
                    nc.sync.dma_start(
                        out_d[qoff:qoff + 128,
                              512 * dc:512 * (dc + 1)], o16[:])
                return run

            def push_tail_hi(p, qc, hi, lhsT_fn, cns):
                """Queue head hi's ctx chains; after hi=1 also transposes
                and (for the last pair) this q-chunk's out-projection.
                Transposes are staggered one qt behind the hi=1 chains so
                the DVE rcp+stt latency is hidden by the next chain's
                matmuls instead of stalling PE."""
                if skip_tail:
                    return
                if hi == 0:
                    for qt in range(4):
                        work_q.append(mk_chain(p, qc, 0, qt, lhsT_fn,
                                               cns[qt]))
                    return
                order = [("c", 0), ("c", 1), ("t", 0), ("c", 2), ("t", 1),
                         ("c", 3), ("t", 2), ("t", 3)]
                for kind, qt in order:
                    if kind == "c":
                        work_q.append(mk_chain(p, qc, 1, qt, lhsT_fn,
                                               cns[qt]))
                    else:
                        work_q.append(mk_transp(p, qc, qt, cns[qt]))
                if p == 3:
                    for qt in range(4):
                        work_q.append(mk_oproj(qc, qt, 0))
                        work_q.append(mk_oproj(qc, qt, 1))

            # ---- inline stage1 schedule: stage1 matmuls/quants are emitted
            # at fixed score-slots inside each unit (deadline-driven), NOT
            # via the work queue -- the queue holds only tail work (chains/
            # transposes/oproj) with a guaranteed ~1-unit lag. ----
            def s1_tile_halves(p, j, n):
                holder = []

                def h0():
                    ps = sh_ps.tile([128, 512], F32, name="s1", tag="sh")
                    holder.append(ps)
                    s1_qk_half(p, j, n, 0, ps)

                def h1():
                    s1_qk_half(p, j, n, 1, holder[0])
                return h0, h1

            inline_work = {u_: {} for u_ in range(16)}

            def add_inline(u_, sl, fn):
                inline_work[u_].setdefault(sl, []).append(fn)

            def add_tile(u_, sl0, sl1, p, j, n):
                h0, h1 = s1_tile_halves(p, j, n)
                add_inline(u_, sl0, h0)
                add_inline(u_, sl1, h1)

            # pair 0 remaining q/k tiles (k chunks feed this unit's own
            # scores -- earliest slots), v-proj for all pairs in unit 0
            add_tile(0, 0, 1, 0, 1, 1)
            add_tile(0, 2, 3, 0, 1, 2)
            add_tile(0, 4, 5, 0, 1, 3)
            add_tile(0, 6, 7, 0, 0, 1)
            add_tile(1, 0, 1, 0, 0, 2)
            add_tile(1, 2, 3, 0, 0, 3)
            if not skip_v:
                for t in range(NSK):
                    add_inline(0, 8 + t, (lambda t=t: s1_v_tile_dr(t))
                               if V_DR else (lambda t=t: s1_v_tile(0, t)))
            # pair p+1 tiles spread over pair p's units
            for p_ in range(3):
                for qc_ in range(4):
                    jn = [(1, 0), (1, 1)] if qc_ == 0 else \
                         [(1, 2), (1, 3)] if qc_ == 1 else \
                         [(0, 0), (0, 1)] if qc_ == 2 else \
                         [(0, 2), (0, 3)]
                    u_ = 4 * p_ + qc_
                    base = 6
                    step = 8
                    if p_ == 0:
                        base = 24 if qc_ == 0 else 8
                        if qc_ == 0:
                            step = 2
                    for (j, n) in jn:
                        add_tile(u_, base, base + 1, p_ + 1, j, n)
                        base += step

            # ---- prologue: k chunk 0 + q chunk 0 of pair 0 inline ----
            s1_qk_tile(0, 1, 0)
            s1_qk_tile(0, 0, 0)

            # ---- main software-pipelined unit loop ----
            for p in range(4):
                for qc in range(4):
                    u = 4 * p + qc
                    last_unit = (u == 15)
                    buf = qk8[p % 2]
                    cns = [ctxN_pool.tile([128, 2, HD], F16, name="cnq",
                                          tag="cn") for _ in range(4)]
                    slot = 0
                    if last_unit:
                        # qt-granular mini-units: the tail of each q-tile
                        # cascades behind its own exps, shrinking the drain
                        for qt in range(4):
                            qoff = 512 * qc + 128 * qt
                            minis = {}
                            for hi in range(2):
                                tiles = []
                                for quarter in range(4):
                                    scp = sc_ps.tile([128, 4, 128], F32,
                                                     name="scm", tag="sc")
                                    for s4 in range(4):
                                        sk = 4 * quarter + s4
                                        nc.tensor.matmul(
                                            scp[:, s4, :],
                                            buf[64 * hi:64 * (hi + 1), 1, :,
                                                128 * sk:128 * (sk + 1)],
                                            buf[64 * hi:64 * (hi + 1), 0, :,
                                                qoff:qoff + 128],
                                            start=True, stop=True,
                                            perf_mode=DR)
                                    et = expT_pool.tile([128, 4, 128], F16,
                                                        name="em", tag="expT")
                                    emit_exp(et[:], scp[:])
                                    tiles.append(et)
                                    pop_work(1)
                                minis[hi] = tiles

                            def mini_fn(minis):
                                def f(hi, qt_, sk):
                                    return minis[hi][sk // 4][:, sk % 4, :]
                                return f
                            lf = mini_fn(minis)
                            work_q.append(mk_chain(p, qc, 0, qt, lf,
                                                    cns[qt]))
                            work_q.append(mk_chain(p, qc, 1, qt, lf,
                                                   cns[qt]))
                            work_q.append(mk_transp(p, qc, qt, cns[qt]))
                            work_q.append(mk_oproj(qc, qt, 0))
                            work_q.append(mk_oproj(qc, qt, 1))
                        continue
                    expTs = {}
                    for hi in range(2):
                        expTs[hi] = expT_pool.tile([128, NSK, 512], F16,
                                                   name=f"e{hi}", tag="expT")
                        for sk in range(NSK):
                            scp = sc_ps.tile([128, 512], F32, name="sc",
                                             tag="sc")
                            nc.tensor.matmul(
                                scp[:],
                                buf[64 * hi:64 * (hi + 1), 1, :,
                                    128 * sk:128 * (sk + 1)],
                                buf[64 * hi:64 * (hi + 1), 0, :,
                                    512 * qc:512 * (qc + 1)],
                                start=True, stop=True, perf_mode=DR)
                            emit_exp(expTs[hi][:, sk, :], scp[:])
                            for fn in inline_work[u].get(slot, []):
                                fn()
                            if u > 0 and slot % 8 in (2, 4, 7):
                                pop_work(1)
                            if p == 3 and slot % 16 == 9:
                                pop_work(1)
                            slot += 1
                        if debug and u == 0 and hi == 0:
                            nc.sync.dma_start(dbg_expT[:], expTs[0][:])
                        def exp_fn(expTs, hi):
                            def f(hi_, qt, sk):
                                return expTs[hi_][:, sk,
                                                  128 * qt:128 * (qt + 1)]
                            return f
                        push_tail_hi(p, qc, hi, exp_fn(expTs, hi), cns)

            # drain
            while work_q:
                work_q.popleft()()
            if debug:
                nc.sync.dma_start(dbg_ctxT[:], ctxT[:])
                nc.sync.dma_start(dbg_v[:], v_sb[:])
                nc.sync.dma_start(dbg_qk8[:], qk8[0][:])

    nc.compile()
    return nc


# ---------------------------------------------------------------------------
# host side: shard, run SPMD, gather
# ---------------------------------------------------------------------------

_RUNNER = None


def _make_runner(nc, n_cores):
    """Jit-once SPMD runner via PJRT (axon)."""
    import jax
    from jax.sharding import Mesh, PartitionSpec
    from jax.experimental.shard_map import shard_map
    from concourse import bass2jax
    from concourse.bass2jax import _bass_exec_p, install_neuronx_cc_hook

    install_neuronx_cc_hook()
    partition_name = nc.partition_id_tensor.name if nc.partition_id_tensor else None

    in_names, out_names, out_avals, zero_outs = [], [], [], []
    for alloc in nc.m.functions[0].allocations:
        if not isinstance(alloc, mybir.MemoryLocationSet):
            continue
        name = alloc.memorylocations[0].name
        if alloc.kind == "ExternalInput":
            if name != partition_name:
                in_names.append(name)
        elif alloc.kind == "ExternalOutput":
            out_names.append(name)
            shape = tuple(alloc.tensor_shape)
            dtype = mybir.dt.np(alloc.dtype)
            out_avals.append(jax.core.ShapedArray(shape, dtype))
            zero_outs.append(np.zeros(shape, dtype))
    n_params = len(in_names)
    n_outs = len(out_avals)
    all_in_names = list(in_names) + list(out_names)
    if partition_name is not None:
        all_in_names.append(partition_name)

    def _body(*args):
        operands = list(args)
        if partition_name is not None:
            operands.append(bass2jax.partition_id_tensor())
        outs = _bass_exec_p.bind(
            *operands,
            out_avals=tuple(out_avals),
            in_names=tuple(all_in_names),
            out_names=tuple(out_names),
            lowering_input_output_aliases=(),
            sim_require_finite=True,
            sim_require_nnan=True,
            nc=nc,
        )
        return tuple(outs)

    devices = jax.devices()[:n_cores]
    mesh = Mesh(np.asarray(devices), ("core",))
    in_specs = (PartitionSpec("core"),) * (n_params + n_outs)
    out_specs = (PartitionSpec("core"),) * n_outs
    jitted = jax.jit(
        shard_map(_body, mesh=mesh, in_specs=in_specs, out_specs=out_specs,
                  check_rep=False),
        keep_unused=True,
    )

    def run(in_maps):
        concat_in = [
            np.concatenate([np.asarray(in_maps[c][n]) for c in range(n_cores)],
                           axis=0)
            for n in in_names
        ]
        concat_zero = [
            np.zeros((n_cores * z.shape[0], *z.shape[1:]), z.dtype)
            for z in zero_outs
        ]
        out_arrs = jitted(*concat_in, *concat_zero)
        jax.block_until_ready(out_arrs)
        return [
            {n: np.asarray(out_arrs[i]).reshape(n_cores, *out_avals[i].shape)[c]
             for i, n in enumerate(out_names)}
            for c in range(n_cores)
        ]

    return run


def _shard_inputs(qkv, W_in, b_in, W_out, b_out):
    """Build the 8 per-core input dicts."""
    f16 = np.float16
    x = np.asarray(qkv, np.float32)
    W_in = np.asarray(W_in, np.float32)
    b_in = np.asarray(b_in, np.float32)
    W_out = np.asarray(W_out, np.float32)
    ident = np.eye(128, dtype=f16)

    f8 = mybir.dt.np(mybir.dt.float8e4)
    in_maps = []
    for c in range(N_CORES):
        b, g = divmod(c, 2)
        qs = slice(512 * g, 512 * (g + 1))
        ks = slice(1024 + 512 * g, 1024 + 512 * (g + 1))
        vs = slice(2048 + 512 * g, 2048 + 512 * (g + 1))
        xT = np.ascontiguousarray(x[b].T)                     # [D, S]
        # wout[p, c_, dout] = W_out[512*g + 128*c_ + p, dout]
        wout = W_out[512 * g:512 * (g + 1), :].reshape(4, 128, D) \
            .transpose(1, 0, 2).astype(f16)
        bqk = np.concatenate([b_in[qs], b_in[ks]]).reshape(8, 128).T \
            .astype(np.float32)
        bqk = np.ascontiguousarray(bqk)
        bv = np.broadcast_to(b_in[vs].reshape(8, HD), (128, 8, HD)) \
            .astype(f16)
        in_map = {
            "wout": wout,
            "bqk": bqk,
            "bv": np.ascontiguousarray(bv),
            "ident": ident,
        }
        if QK_DR or V_DR:
            # x8[p, pl, kc, s] = xT[128*kc + 64*pl + p, s]
            in_map["x8"] = np.ascontiguousarray(
                xT.reshape(8, 2, 64, S).transpose(2, 1, 0, 3).astype(f8))
        if not (QK_DR and V_DR):
            # x16[p, kc, s] = xT[128*kc+p, s]
            in_map["x16"] = xT.reshape(8, 128, S).transpose(1, 0, 2) \
                .astype(f16)
        if QK_DR:
            wqk_full = np.concatenate([W_in[:, qs], W_in[:, ks]],
                                      axis=1) * WSCL            # [D, 1024]
            in_map["wqk8"] = np.ascontiguousarray(
                wqk_full.reshape(8, 2, 64, 1024).transpose(2, 1, 0, 3)
                .astype(f8))
        else:
            # wqk[p, kc, f]: f 0..511 q feats, 512..1023 k feats
            wq = W_in[:, qs].reshape(8, 128, 512).transpose(1, 0, 2)
            wk = W_in[:, ks].reshape(8, 128, 512).transpose(1, 0, 2)
            in_map["wqk"] = np.concatenate([wq, wk], axis=2).astype(f16)
        if V_DR:
            in_map["wv8"] = np.ascontiguousarray(
                (W_in[:, vs] * WSCL).reshape(8, 2, 64, 512)
                .transpose(2, 1, 0, 3).astype(f8))
        else:
            in_map["wv"] = W_in[:, vs].reshape(8, 128, 512) \
                .transpose(1, 0, 2).astype(f16)
        in_maps.append(in_map)
    return in_maps


def kernel(qkv, W_in, b_in, W_out, b_out):
    global _RUNNER
    if _RUNNER is None:
        nc = build_nc()
        _RUNNER = _make_runner(nc, N_CORES)
    in_maps = _shard_inputs(qkv, W_in, b_in, W_out, b_out)
    results = _RUNNER(in_maps)
    b_out = np.asarray(b_out, np.float32)
    out = np.empty((B, S, D), np.float32)
    for b in range(B):
        out[b] = (results[2 * b]["out"].astype(np.float32)
                  + results[2 * b + 1]["out"].astype(np.float32) + b_out)
    return out


if __name__ == "__main__":
    rng = np.random.default_rng(0)
    qkv = rng.standard_normal((B, S, D)).astype(np.float32)
    sc = 1.0 / np.sqrt(D)
    W_in = rng.uniform(-sc, sc, (D, 3 * D)).astype(np.float32)
    b_in = rng.uniform(-sc, sc, (3 * D,)).astype(np.float32)
    W_out = rng.uniform(-sc, sc, (D, D)).astype(np.float32)
    b_out = rng.uniform(-sc, sc, (D,)).astype(np.float32)
    got = kernel(qkv, W_in, b_in, W_out, b_out)
    print("kernel ran, output shape", got.shape)



# revision 48
# speedup vs baseline: 1.1284x; 1.1240x over previous
"""Multi-head self-attention on 8 TRN2 NeuronCores.

Sharding: core c -> (batch b = c//2, head-half g = c%2, i.e. 8 of 16 heads).
Each core computes qkv-proj + attention + out-proj partial for its 8 heads;
host sums the two partials per batch and adds b_out.

Design (v2):
- stage1 q,k,v projections in fp16 (1 cyc/row), q/k results quantized to
  fp8e4 on the PSUM->SBUF copy (bias added via per-partition tensor_scalar).
- scores as zero-padded DoubleRow fp8 matmuls (0.5 cyc/row): operands
  [64, 2, *] with the i=1 plane zeroed; out tile [128 k-pos, 512 q-pos].
- exp on ACT (scale=0.125 applied in the activation), fp16 out.
- ctx computed transposed: out [128 q, 65] with lhsT = exp tile (stationary)
  and rhs = V' [128 k, 65] whose 65th column is ones -> denominator lands in
  out[:, 64] = per-partition scalar. Normalization + V-bias is then a single
  scalar_tensor_tensor (mult, add) per (head, q-tile).
- ctx^T via PE transpose (fp16, identity rhs) packing head pairs into
  [128, 128] PSUM tiles; out-projection over the packed [d, q] layout,
  fp16 output DMA'd per tile; host sums core pairs + b_out.
- software pipelining: unit (pair, qc) emits its 32 score matmuls + 16 exps,
  then the previous unit's ctx/norm/transpose tail, then next-pair stage1
  or out-projection work, keeping ACT (the bottleneck) saturated.
"""
import sys
sys.path.insert(0, '/opt/trn_rl_repo')

import numpy as np

import concourse.bass as bass
import concourse.mybir as mybir
import concourse.tile as tile
from concourse import bacc

F32 = mybir.dt.float32
F16 = mybir.dt.float16
F8E4 = mybir.dt.float8e4
I16 = mybir.dt.int16
DR = mybir.MatmulPerfMode.DoubleRow
Exp = mybir.ActivationFunctionType.Exp
MULT = mybir.AluOpType.mult
ADD = mybir.AluOpType.add

B, S, D = 4, 2048, 1024
H, HD = 16, 64
N_CORES = 8
NSK = S // 128            # 16 k-chunks of 128
NQT = S // 128            # 16 q-tiles of 128
SCALE = 0.125             # 1/sqrt(HD)

# Schraudolph exp for DVE/GPSIMD offload: fp16 bits = trunc(ALPHA*s + BETA)
# approximates exp(s * SCALE) to ~3% max rel error (error cancels partially
# in the softmax ratio). ALPHA = 1024*log2(e)*SCALE.
ALPHA = 184.6649652337873
BETA = 15316.431477991726

# per-unit exp engine assignment (16 slots): A=ACT native exp,
# D=DVE Schraudolph, P=GPSIMD Schraudolph
# 32 slots, 16A/9D/7P: balances ACT/DVE/GPSIMD busy at ~155-165us each
EXP_PATTERN = "ADPAADPAADPAADAPADPAADPAADPAADAD"

# stage1 projections as fp8 DoubleRow matmuls (halves PE cost of each).
# Weights are pre-scaled by WSCL on the host so W_in/W_out values
# (~U(-1/32,1/32)) sit mid-range in fp8e4m3 instead of subnormal;
# compensated by 1/WSCL on the PSUM->SBUF copies.
QK_DR = True
V_DR = True
WSCL = 64.0

# engine assignment knobs (sweepable): which engine runs each helper op
QUANT_ON = "vector"   # stage1 q/k fp8 quant: gpsimd | vector | scalar
VCOPY_ON = "vector"   # v psum->sbuf scaled copy: gpsimd | vector
STT_ON = "gpsimd"     # ctx normalize (scalar_tensor_tensor): vector | gpsimd
OCOPY_ON = "vector"   # oproj psum->sbuf copy: vector | gpsimd
OPROJ_PS = "sh"       # oproj psum tag: sh | sc
SH_BUFS = 2           # sh (stage1/oproj) psum bufs
SC_BUFS = 4           # score psum bufs
TP_PS = "ctx"         # transpose psum tag: sh | ctx
V_SPLIT = True        # v-proj per head-pair tiles (spread over units)


def build_nc(skip_tail=False, skip_exp=False, sc_bufs=4, skip_v=False, fake_in=False, debug=False):
    nc = bacc.Bacc(None, target_bir_lowering=False)

    if QK_DR or V_DR:
        x8_d = nc.dram_tensor("x8", [64, 2, 8, S], F8E4, kind="ExternalInput")
    if not (QK_DR and V_DR):
        x16_d = nc.dram_tensor("x16", [128, 8, S], F16, kind="ExternalInput")
    if QK_DR:
        wqk8_d = nc.dram_tensor("wqk8", [64, 2, 8, 1024], F8E4,
                                kind="ExternalInput")
    else:
        wqk_d = nc.dram_tensor("wqk", [128, 8, 1024], F16,
                               kind="ExternalInput")
    if V_DR:
        wv8_d = nc.dram_tensor("wv8", [64, 2, 8, 512], F8E4,
                               kind="ExternalInput")
    else:
        wv_d = nc.dram_tensor("wv", [128, 8, 512], F16, kind="ExternalInput")
    wout_d = nc.dram_tensor("wout", [128, 4, D], F16, kind="ExternalInput")
    bqk_d = nc.dram_tensor("bqk", [128, 8], F32, kind="ExternalInput")
    bv_d = nc.dram_tensor("bv", [128, 8, HD], F16, kind="ExternalInput")
    ident_d = nc.dram_tensor("ident", [128, 128], F16, kind="ExternalInput")
    out_d = nc.dram_tensor("out", [S, D], F16, kind="ExternalOutput")
    if debug:
        dbg_qk8 = nc.dram_tensor("dbg_qk8", [128, 2, 2, S], F8E4,
                                 kind="ExternalOutput")
        dbg_expT = nc.dram_tensor("dbg_expT", [128, NSK, 512], F16,
                                  kind="ExternalOutput")
        dbg_ctxT = nc.dram_tensor("dbg_ctxT", [128, 4, S], F16,
                                  kind="ExternalOutput")
        dbg_v = nc.dram_tensor("dbg_v", [128, NSK, 8, HD + 1], F16,
                               kind="ExternalOutput")

    with tile.TileContext(nc) as tc:
        with (
            tc.tile_pool(name="const", bufs=1) as cpool,
            tc.tile_pool(name="expT", bufs=4) as expT_pool,
            tc.tile_pool(name="ctxN", bufs=4) as ctxN_pool,
            tc.tile_pool(name="rcp", bufs=4) as rcp_pool,
            tc.tile_pool(name="osb", bufs=2) as out_pool,
            tc.tile_pool(name="scps", bufs=SC_BUFS, space="PSUM") as sc_ps,
            tc.tile_pool(name="ctxps", bufs=2, space="PSUM") as ctx_ps,
            tc.tile_pool(name="shps", bufs=SH_BUFS, space="PSUM") as sh_ps,
        ):
            # ---- constants / persistent tiles (DMAs ordered so the
            # prologue's pair-0 stage1 work can start immediately) ----
            if QK_DR or V_DR:
                x8 = cpool.tile([64, 2, 8, S], F8E4)
            if not (QK_DR and V_DR):
                x16 = cpool.tile([128, 8, S], F16)
            if QK_DR:
                wqk8 = cpool.tile([64, 2, 8, 1024], F8E4)
            else:
                wqk = cpool.tile([128, 8, 1024], F16)
            bqk = cpool.tile([128, 8], F32)
            if V_DR:
                wv8 = cpool.tile([64, 2, 8, 512], F8E4)
            else:
                wv = cpool.tile([128, 8, 512], F16)
            wout = cpool.tile([128, 4, D], F16)
            bv = cpool.tile([128, 8, HD], F16)
            ident = cpool.tile([128, 128], F16)
            # critical-path DMAs on SP, halves sized to unblock the first
            # stage1 half-tile ASAP; bulk loads ride the ACT hwdge queue
            nc.sync.dma_start(bqk[:], bqk_d[:])
            if QK_DR:
                nc.sync.dma_start(wqk8[:, :, :, 512:640], wqk8_d[:, :, :, 512:640])
            else:
                nc.sync.dma_start(wqk[:, :, 512:640], wqk_d[:, :, 512:640])
            if QK_DR or V_DR:
                nc.sync.dma_start(x8[:, :, :, 0:512], x8_d[:, :, :, 0:512])
            if not (QK_DR and V_DR):
                nc.sync.dma_start(x16[:, :, 0:512], x16_d[:, :, 0:512])
            if QK_DR:
                nc.sync.dma_start(wqk8[:, :, :, 0:128], wqk8_d[:, :, :, 0:128])
            else:
                nc.sync.dma_start(wqk[:, :, 0:128], wqk_d[:, :, 0:128])
            for n in range(1, 4):
                if QK_DR or V_DR:
                    nc.sync.dma_start(x8[:, :, :, 512 * n:512 * (n + 1)],
                                      x8_d[:, :, :, 512 * n:512 * (n + 1)])
                if not (QK_DR and V_DR):
                    nc.sync.dma_start(x16[:, :, 512 * n:512 * (n + 1)],
                                      x16_d[:, :, 512 * n:512 * (n + 1)])
            if V_DR:
                nc.sync.dma_start(wv8[:], wv8_d[:])
            else:
                nc.sync.dma_start(wv[:], wv_d[:])
            nc.sync.dma_start(bv[:], bv_d[:])
            nc.sync.dma_start(ident[:], ident_d[:])
            for p_ in range(1, 4):
                if QK_DR:
                    nc.sync.dma_start(
                        wqk8[:, :, :, 512 + 128 * p_:640 + 128 * p_],
                        wqk8_d[:, :, :, 512 + 128 * p_:640 + 128 * p_])
                    nc.sync.dma_start(
                        wqk8[:, :, :, 128 * p_:128 * (p_ + 1)],
                        wqk8_d[:, :, :, 128 * p_:128 * (p_ + 1)])
                else:
                    nc.sync.dma_start(
                        wqk[:, :, 512 + 128 * p_:640 + 128 * p_],
                        wqk_d[:, :, 512 + 128 * p_:640 + 128 * p_])
                    nc.sync.dma_start(wqk[:, :, 128 * p_:128 * (p_ + 1)],
                                      wqk_d[:, :, 128 * p_:128 * (p_ + 1)])
            nc.sync.dma_start(wout[:], wout_d[:])

            # PE p-state warm-up: the cost model charges cold-clock cycles
            # until the PE has been continuously busy for 3us at prep time.
            # Run a dependency-free accumulating matmul chain from t=0 so the
            # real prologue matmuls are prepped at the full 2.4 GHz clock.
            wrm = cpool.tile([128, 512], F16)
            nc.vector.memset(wrm[:], 0.125)
            # preload the ACT exp table at t~0 (1.3us) so the first real
            # exp doesn't eat the table-load latency mid-pipeline
            actwarm = cpool.tile([128, 1], F16)
            nc.scalar.activation(actwarm[:], wrm[:, 0:1], Exp, scale=1.0)
            wps = sh_ps.tile([128, 512], F32, name="wps", tag="sh")
            NWARM = 14
            for i in range(NWARM):
                nc.tensor.matmul(wps[:], wrm[:, 0:128], wrm[:],
                                 start=(i == 0), stop=(i == NWARM - 1))

            # V': [s%128, sk, head, hd+1]; [..,64] = 1.0 for denominators
            v_sb = cpool.tile([128, NSK, 8, HD + 1], F16)
            nc.vector.memset(v_sb[:, :, :, HD], 1.0)

            # q/k fp8 double-buffered (ping-pong by pair parity).
            # dims: [part(=hd within head pair), qk, i(double-row), s]
            # QK_DR: both DR planes carry the same data (score doubles;
            # compensated in the exp scale) -> no zero-plane memset needed.
            # Legacy path: i=1 plane zeroed once on DVE.
            qk8 = [cpool.tile([128, 2, 2, S], F8E4, name=f"qk8_{b_}")
                   for b_ in range(2)]
            if not QK_DR:
                for b_ in range(2):
                    nc.vector.memset(qk8[b_][:], 0.0)

            # effective exp scale: scores are doubled by the replicated DR
            # planes when QK_DR is on
            esc = SCALE * (0.5 if QK_DR else 1.0)
            alpha = 1477.3193223344908 * esc

            # exp dispatch: round-robin over ACT/DVE/GPSIMD per EXP_PATTERN
            exp_ctr = [0]

            def emit_exp(dst, src):
                kind = EXP_PATTERN[exp_ctr[0] % len(EXP_PATTERN)]
                exp_ctr[0] += 1
                if skip_exp:
                    nc.vector.tensor_copy(dst, src)
                elif kind == 'A':
                    nc.scalar.activation(dst, src, Exp, scale=esc)
                elif kind == 'D':
                    nc.vector.tensor_scalar(dst.bitcast(I16), src,
                                            alpha, BETA, op0=MULT, op1=ADD)
                else:
                    nc.gpsimd.tensor_scalar(dst.bitcast(I16), src,
                                            alpha, BETA, op0=MULT, op1=ADD)

            # ctx^T accumulator [d-part packed by pair, pair, q] fp16
            ctxT = cpool.tile([128, 4, S], F16)

            # ---- stage1 helpers ----
            def s1_qk_half(p, j, n, half, ps):
                """half of the contraction for pair p, j=0 q/1 k, chunk n."""
                foff = 128 * p + 512 * j
                for kc in range(4 * half, 4 * half + 4):
                    if QK_DR:
                        nc.tensor.matmul(
                            ps[:], wqk8[:, :, kc, foff:foff + 128],
                            x8[:, :, kc, 512 * n:512 * (n + 1)],
                            start=(kc == 0), stop=(kc == 7), perf_mode=DR)
                    else:
                        nc.tensor.matmul(
                            ps[:], wqk[:, kc, foff:foff + 128],
                            x16[:, kc, 512 * n:512 * (n + 1)],
                            start=(kc == 0), stop=(kc == 7))
                if half == 1:
                    if QK_DR:
                        bias = bqk[:, 4 * j + p:4 * j + p + 1]
                        for pl in range(2):
                            dst = qk8[p % 2][:, j, pl,
                                             512 * n:512 * (n + 1)]
                            if QUANT_ON == "scalar":
                                nc.scalar.activation(
                                    dst, ps[:],
                                    mybir.ActivationFunctionType.Identity,
                                    bias=bias, scale=1.0 / WSCL)
                            else:
                                qeng = (nc.gpsimd if QUANT_ON == "gpsimd"
                                        else nc.vector)
                                qeng.tensor_scalar(
                                    dst, ps[:], 1.0 / WSCL, bias,
                                    op0=MULT, op1=ADD)
                    else:
                        nc.gpsimd.tensor_scalar_add(
                            qk8[p % 2][:, j, 0, 512 * n:512 * (n + 1)], ps[:],
                            bqk[:, 4 * j + p:4 * j + p + 1])

            def s1_qk_tile(p, j, n):
                ps = sh_ps.tile([128, 512], F32, name="s1", tag="sh")
                s1_qk_half(p, j, n, 0, ps)
                s1_qk_half(p, j, n, 1, ps)

            def s1_qk_items(p, j, n):
                ps = sh_ps.tile([128, 512], F32, name="s1", tag="sh")
                return [(lambda: s1_qk_half(p, j, n, 0, ps)),
                        (lambda: s1_qk_half(p, j, n, 1, ps))]

            def s1_v_tile(p, t):
                """v for pair p (128 feats), seq tile t (128 rows)."""
                ps = sh_ps.tile([128, 128], F32, name="s1v", tag="sh")
                for kc in range(8):
                    nc.tensor.matmul(
                        ps[:], x16[:, kc, 128 * t:128 * (t + 1)],
                        wv[:, kc, 128 * p:128 * (p + 1)],
                        start=(kc == 0), stop=(kc == 7))
                nc.gpsimd.tensor_copy(
                    v_sb[:, t, 2 * p:2 * p + 2, 0:HD],
                    ps.rearrange("a (h d) -> a h d", h=2))

            def s1_v_tile_dr(t):
                """v for ALL 8 heads (512 feats), seq tile t, fp8 DoubleRow."""
                ps = sh_ps.tile([128, 512], F32, name="s1v", tag="sh")
                for kc in range(8):
                    nc.tensor.matmul(
                        ps[:], x8[:, :, kc, 128 * t:128 * (t + 1)],
                        wv8[:, :, kc, :],
                        start=(kc == 0), stop=(kc == 7), perf_mode=DR)
                veng = nc.gpsimd if VCOPY_ON == "gpsimd" else nc.vector
                veng.tensor_scalar(
                    v_sb[:, t, :, 0:HD],
                    ps.rearrange("a (h d) -> a h d", h=8),
                    1.0 / WSCL, None, op0=MULT)

            def s1_v_pair_dr(p, t):
                """v for one head pair (128 feats), seq tile t, fp8 DR."""
                ps = sh_ps.tile([128, 128], F32, name="s1vp", tag="sh")
                for kc in range(8):
                    nc.tensor.matmul(
                        ps[:], x8[:, :, kc, 128 * t:128 * (t + 1)],
                        wv8[:, :, kc, 128 * p:128 * (p + 1)],
                        start=(kc == 0), stop=(kc == 7), perf_mode=DR)
                veng = nc.gpsimd if VCOPY_ON == "gpsimd" else nc.vector
                veng.tensor_scalar(
                    v_sb[:, t, 2 * p:2 * p + 2, 0:HD],
                    ps.rearrange("a (h d) -> a h d", h=2),
                    1.0 / WSCL, None, op0=MULT)

            # ---- deferred PE work queue: popped between score/exp pairs so
            # the ACT engine (bottleneck) never starves while PE does the
            # ctx/transpose/stage1/outproj work of earlier units ----
            from collections import deque
            work_q = deque()

            def pop_work(k):
                for _ in range(k):
                    if work_q:
                        work_q.popleft()()

            def mk_chain(p, qc, hi, qt, lhsT_fn, cn):
                def run():
                    h = 2 * p + hi
                    cps = ctx_ps.tile([128, HD + 1], F32,
                                      name="ctx", tag="ctx")
                    for sk in range(NSK):
                        nc.tensor.matmul(
                            cps[:],
                            lhsT_fn(hi, qt, sk),
                            v_sb[:, sk, h, :],
                            start=(sk == 0), stop=(sk == NSK - 1))
                    rcp = rcp_pool.tile([128, 1], F32, name="rc",
                                        tag="rc")
                    nc.vector.reciprocal_approx_fast(
                        rcp[:], cps[:, HD:HD + 1])
                    seng = nc.vector if STT_ON == "vector" else nc.gpsimd
                    seng.scalar_tensor_tensor(
                        cn[:, hi, :], cps[:, 0:HD], rcp[:], bv[:, h, :],
                        op0=MULT, op1=ADD)
                return run

            def mk_transp(p, qc, qt, cn):
                def run():
                    tp_pool = sh_ps if TP_PS == "sh" else ctx_ps
                    tp = tp_pool.tile([128, 128], F16, name="tp",
                                      tag=("sh" if TP_PS == "sh" else "ctx"))
                    nc.tensor.matmul(
                        tp[0:64, :], cn[:, 0, :], ident[:],
                        start=True, stop=True, is_transpose=True,
                        tile_position=(0, 0))
                    nc.tensor.matmul(
                        tp[64:128, :], cn[:, 1, :], ident[:],
                        start=True, stop=True, is_transpose=True,
                        tile_position=(0, 64))
                    qoff = 512 * qc + 128 * qt
                    nc.vector.tensor_copy(
                        ctxT[:, p, qoff:qoff + 128], tp[:])
                return run

            def mk_oproj(qc, qt, dc):
                def run():
                    qoff = 512 * qc + 128 * qt
                    op_pool = sh_ps if OPROJ_PS == "sh" else sc_ps
                    ops = op_pool.tile([128, 512], F32, name="op",
                                       tag=("sh" if OPROJ_PS == "sh"
                                            else "sc"))
                    for c in range(4):
                        nc.tensor.matmul(
                            ops[:], ctxT[:, c, qoff:qoff + 128],
                            wout[:, c, 512 * dc:512 * (dc + 1)],
                            start=(c == 0), stop=(c == 3))
                    o16 = out_pool.tile([128, 512], F16, name="o")
                    oeng = nc.vector if OCOPY_ON == "vector" else nc.gpsimd
                    oeng.tensor_copy(o16[:], ops[:])
                    nc.sync.dma_start(
                        out_d[qoff:qoff + 128,
                              512 * dc:512 * (dc + 1)], o16[:])
                return run

            def push_tail_hi(p, qc, hi, lhsT_fn, cns):
                """Queue head hi's ctx chains; after hi=1 also transposes
                and (for the last pair) this q-chunk's out-projection.
                Transposes are staggered one qt behind the hi=1 chains so
                the DVE rcp+stt latency is hidden by the next chain's
                matmuls instead of stalling PE."""
                if skip_tail:
                    return
                if hi == 0:
                    for qt in range(4):
                        work_q.append(mk_chain(p, qc, 0, qt, lhsT_fn,
                                               cns[qt]))
                    return
                order = [("c", 0), ("c", 1), ("t", 0), ("c", 2), ("t", 1),
                         ("c", 3), ("t", 2), ("t", 3)]
                for kind, qt in order:
                    if kind == "c":
                        work_q.append(mk_chain(p, qc, 1, qt, lhsT_fn,
                                               cns[qt]))
                    else:
                        work_q.append(mk_transp(p, qc, qt, cns[qt]))
                if p == 3:
                    for qt in range(4):
                        work_q.append(mk_oproj(qc, qt, 0))
                        work_q.append(mk_oproj(qc, qt, 1))

            # ---- inline stage1 schedule: stage1 matmuls/quants are emitted
            # at fixed score-slots inside each unit (deadline-driven), NOT
            # via the work queue -- the queue holds only tail work (chains/
            # transposes/oproj) with a guaranteed ~1-unit lag. ----
            def s1_tile_halves(p, j, n):
                holder = []

                def h0():
                    ps = sh_ps.tile([128, 512], F32, name="s1", tag="sh")
                    holder.append(ps)
                    s1_qk_half(p, j, n, 0, ps)

                def h1():
                    s1_qk_half(p, j, n, 1, holder[0])
                return h0, h1

            inline_work = {u_: {} for u_ in range(16)}

            def add_inline(u_, sl, fn):
                inline_work[u_].setdefault(sl, []).append(fn)

            def add_tile(u_, sl0, sl1, p, j, n):
                h0, h1 = s1_tile_halves(p, j, n)
                add_inline(u_, sl0, h0)
                add_inline(u_, sl1, h1)

            # pair 0 remaining q/k tiles (k chunks feed this unit's own
            # scores -- earliest slots), v-proj for all pairs in unit 0
            add_tile(0, 0, 1, 0, 1, 1)
            add_tile(0, 2, 3, 0, 1, 2)
            add_tile(0, 4, 5, 0, 1, 3)
            add_tile(0, 6, 7, 0, 0, 1)
            add_tile(1, 0, 1, 0, 0, 2)
            add_tile(1, 2, 3, 0, 0, 3)
            if not skip_v:
                if V_DR and V_SPLIT:
                    # pair-p v tiles land just before pair p's units
                    for t in range(NSK):
                        add_inline(0, 8 + t, lambda t=t: s1_v_pair_dr(0, t))
                    for p_v in range(1, 4):
                        for t in range(NSK):
                            u_v = 4 * (p_v - 1) + 2 + t // 8
                            add_inline(u_v, 18 + t % 8,
                                       lambda p_v=p_v, t=t:
                                       s1_v_pair_dr(p_v, t))
                elif V_DR:
                    for t in range(NSK):
                        add_inline(0, 8 + t, lambda t=t: s1_v_tile_dr(t))
                else:
                    for t in range(NSK):
                        add_inline(0, 8 + t, lambda t=t: s1_v_tile(0, t))
            # pair p+1 tiles spread over pair p's units
            for p_ in range(3):
                for qc_ in range(4):
                    jn = [(1, 0), (1, 1)] if qc_ == 0 else \
                         [(1, 2), (1, 3)] if qc_ == 1 else \
                         [(0, 0), (0, 1)] if qc_ == 2 else \
                         [(0, 2), (0, 3)]
                    u_ = 4 * p_ + qc_
                    base = 6
                    step = 8
                    if p_ == 0:
                        base = 24 if qc_ == 0 else 8
                        if qc_ == 0:
                            step = 2
                    for (j, n) in jn:
                        add_tile(u_, base, base + 1, p_ + 1, j, n)
                        base += step

            # ---- prologue: k chunk 0 + q chunk 0 of pair 0 inline ----
            s1_qk_tile(0, 1, 0)
            s1_qk_tile(0, 0, 0)

            # ---- main software-pipelined unit loop ----
            for p in range(4):
                for qc in range(4):
                    u = 4 * p + qc
                    last_unit = (u == 15)
                    buf = qk8[p % 2]
                    cns = [ctxN_pool.tile([128, 2, HD], F16, name="cnq",
                                          tag="cn") for _ in range(4)]
                    slot = 0
                    if last_unit:
                        # qt-granular mini-units: the tail of each q-tile
                        # cascades behind its own exps, shrinking the drain
                        for qt in range(4):
                            qoff = 512 * qc + 128 * qt
                            minis = {}
                            for hi in range(2):
                                tiles = []
                                for quarter in range(4):
                                    scp = sc_ps.tile([128, 4, 128], F32,
                                                     name="scm", tag="sc")
                                    for s4 in range(4):
                                        sk = 4 * quarter + s4
                                        nc.tensor.matmul(
                                            scp[:, s4, :],
                                            buf[64 * hi:64 * (hi + 1), 1, :,
                                                128 * sk:128 * (sk + 1)],
                                            buf[64 * hi:64 * (hi + 1), 0, :,
                                                qoff:qoff + 128],
                                            start=True, stop=True,
                                            perf_mode=DR)
                                    et = expT_pool.tile([128, 4, 128], F16,
                                                        name="em", tag="expT")
                                    emit_exp(et[:], scp[:])
                                    tiles.append(et)
                                    pop_work(1)
                                minis[hi] = tiles

                            def mini_fn(minis):
                                def f(hi, qt_, sk):
                                    return minis[hi][sk // 4][:, sk % 4, :]
                                return f
                            lf = mini_fn(minis)
                            work_q.append(mk_chain(p, qc, 0, qt, lf,
                                                    cns[qt]))
                            work_q.append(mk_chain(p, qc, 1, qt, lf,
                                                   cns[qt]))
                            work_q.append(mk_transp(p, qc, qt, cns[qt]))
                            work_q.append(mk_oproj(qc, qt, 0))
                            work_q.append(mk_oproj(qc, qt, 1))
                        continue
                    expTs = {}
                    for hi in range(2):
                        expTs[hi] = expT_pool.tile([128, NSK, 512], F16,
                                                   name=f"e{hi}", tag="expT")
                        for sk in range(NSK):
                            scp = sc_ps.tile([128, 512], F32, name="sc",
                                             tag="sc")
                            nc.tensor.matmul(
                                scp[:],
                                buf[64 * hi:64 * (hi + 1), 1, :,
                                    128 * sk:128 * (sk + 1)],
                                buf[64 * hi:64 * (hi + 1), 0, :,
                                    512 * qc:512 * (qc + 1)],
                                start=True, stop=True, perf_mode=DR)
                            emit_exp(expTs[hi][:, sk, :], scp[:])
                            for fn in inline_work[u].get(slot, []):
                                fn()
                            if u > 0 and slot % 8 in (2, 4, 7):
                                pop_work(1)
                            if p == 3 and slot % 4 == 1:
                                pop_work(1)
                            slot += 1
                        if debug and u == 0 and hi == 0:
                            nc.sync.dma_start(dbg_expT[:], expTs[0][:])
                        def exp_fn(expTs, hi):
                            def f(hi_, qt, sk):
                                return expTs[hi_][:, sk,
                                                  128 * qt:128 * (qt + 1)]
                            return f
                        push_tail_hi(p, qc, hi, exp_fn(expTs, hi), cns)

            # drain
            while work_q:
                work_q.popleft()()
            if debug:
                nc.sync.dma_start(dbg_ctxT[:], ctxT[:])
                nc.sync.dma_start(dbg_v[:], v_sb[:])
                nc.sync.dma_start(dbg_qk8[:], qk8[0][:])

    nc.compile()
    return nc


# ---------------------------------------------------------------------------
# host side: shard, run SPMD, gather
# ---------------------------------------------------------------------------

_RUNNER = None


def _make_runner(nc, n_cores):
    """Jit-once SPMD runner via PJRT (axon)."""
    import jax
    from jax.sharding import Mesh, PartitionSpec
    from jax.experimental.shard_map import shard_map
    from concourse import bass2jax
    from concourse.bass2jax import _bass_exec_p, install_neuronx_cc_hook

    install_neuronx_cc_hook()
    partition_name = nc.partition_id_tensor.name if nc.partition_id_tensor else None

    in_names, out_names, out_avals, zero_outs = [], [], [], []
    for alloc in nc.m.functions[0].allocations:
        if not isinstance(alloc, mybir.MemoryLocationSet):
            continue
        name = alloc.memorylocations[0].name
        if alloc.kind == "ExternalInput":
            if name != partition_name:
                in_names.append(name)
        elif alloc.kind == "ExternalOutput":
            out_names.append(name)
            shape = tuple(alloc.tensor_shape)
            dtype = mybir.dt.np(alloc.dtype)
            out_avals.append(jax.core.ShapedArray(shape, dtype))
            zero_outs.append(np.zeros(shape, dtype))
    n_params = len(in_names)
    n_outs = len(out_avals)
    all_in_names = list(in_names) + list(out_names)
    if partition_name is not None:
        all_in_names.append(partition_name)

    def _body(*args):
        operands = list(args)
        if partition_name is not None:
            operands.append(bass2jax.partition_id_tensor())
        outs = _bass_exec_p.bind(
            *operands,
            out_avals=tuple(out_avals),
            in_names=tuple(all_in_names),
            out_names=tuple(out_names),
            lowering_input_output_aliases=(),
            sim_require_finite=True,
            sim_require_nnan=True,
            nc=nc,
        )
        return tuple(outs)

    devices = jax.devices()[:n_cores]
    mesh = Mesh(np.asarray(devices), ("core",))
    in_specs = (PartitionSpec("core"),) * (n_params + n_outs)
    out_specs = (PartitionSpec("core"),) * n_outs
    jitted = jax.jit(
        shard_map(_body, mesh=mesh, in_specs=in_specs, out_specs=out_specs,
                  check_rep=False),
        keep_unused=True,
    )

    def run(in_maps):
        concat_in = [
            np.concatenate([np.asarray(in_maps[c][n]) for c in range(n_cores)],
                           axis=0)
            for n in in_names
        ]
        concat_zero = [
            np.zeros((n_cores * z.shape[0], *z.shape[1:]), z.dtype)
            for z in zero_outs
        ]
        out_arrs = jitted(*concat_in, *concat_zero)
        jax.block_until_ready(out_arrs)
        return [
            {n: np.asarray(out_arrs[i]).reshape(n_cores, *out_avals[i].shape)[c]
             for i, n in enumerate(out_names)}
            for c in range(n_cores)
        ]

    return run


def _shard_inputs(qkv, W_in, b_in, W_out, b_out):
    """Build the 8 per-core input dicts."""
    f16 = np.float16
    x = np.asarray(qkv, np.float32)
    W_in = np.asarray(W_in, np.float32)
    b_in = np.asarray(b_in, np.float32)
    W_out = np.asarray(W_out, np.float32)
    ident = np.eye(128, dtype=f16)

    f8 = mybir.dt.np(mybir.dt.float8e4)
    in_maps = []
    for c in range(N_CORES):
        b, g = divmod(c, 2)
        qs = slice(512 * g, 512 * (g + 1))
        ks = slice(1024 + 512 * g, 1024 + 512 * (g + 1))
        vs = slice(2048 + 512 * g, 2048 + 512 * (g + 1))
        xT = np.ascontiguousarray(x[b].T)                     # [D, S]
        # wout[p, c_, dout] = W_out[512*g + 128*c_ + p, dout]
        wout = W_out[512 * g:512 * (g + 1), :].reshape(4, 128, D) \
            .transpose(1, 0, 2).astype(f16)
        bqk = np.concatenate([b_in[qs], b_in[ks]]).reshape(8, 128).T \
            .astype(np.float32)
        bqk = np.ascontiguousarray(bqk)
        bv = np.broadcast_to(b_in[vs].reshape(8, HD), (128, 8, HD)) \
            .astype(f16)
        in_map = {
            "wout": wout,
            "bqk": bqk,
            "bv": np.ascontiguousarray(bv),
            "ident": ident,
        }
        if QK_DR or V_DR:
            # x8[p, pl, kc, s] = xT[128*kc + 64*pl + p, s]
            in_map["x8"] = np.ascontiguousarray(
                xT.reshape(8, 2, 64, S).transpose(2, 1, 0, 3).astype(f8))
        if not (QK_DR and V_DR):
            # x16[p, kc, s] = xT[128*kc+p, s]
            in_map["x16"] = xT.reshape(8, 128, S).transpose(1, 0, 2) \
                .astype(f16)
        if QK_DR:
            wqk_full = np.concatenate([W_in[:, qs], W_in[:, ks]],
                                      axis=1) * WSCL            # [D, 1024]
            in_map["wqk8"] = np.ascontiguousarray(
                wqk_full.reshape(8, 2, 64, 1024).transpose(2, 1, 0, 3)
                .astype(f8))
        else:
            # wqk[p, kc, f]: f 0..511 q feats, 512..1023 k feats
            wq = W_in[:, qs].reshape(8, 128, 512).transpose(1, 0, 2)
            wk = W_in[:, ks].reshape(8, 128, 512).transpose(1, 0, 2)
            in_map["wqk"] = np.concatenate([wq, wk], axis=2).astype(f16)
        if V_DR:
            in_map["wv8"] = np.ascontiguousarray(
                (W_in[:, vs] * WSCL).reshape(8, 2, 64, 512)
                .transpose(2, 1, 0, 3).astype(f8))
        else:
            in_map["wv"] = W_in[:, vs].reshape(8, 128, 512) \
                .transpose(1, 0, 2).astype(f16)
        in_maps.append(in_map)
    return in_maps


def kernel(qkv, W_in, b_in, W_out, b_out):
    global _RUNNER
    if _RUNNER is None:
        nc = build_nc()
        _RUNNER = _make_runner(nc, N_CORES)
    in_maps = _shard_inputs(qkv, W_in, b_in, W_out, b_out)
    results = _RUNNER(in_maps)
    b_out = np.asarray(b_out, np.float32)
    out = np.empty((B, S, D), np.float32)
    for b in range(B):
        out[b] = (results[2 * b]["out"].astype(np.float32)
                  + results[2 * b + 1]["out"].astype(np.float32) + b_out)
    return out


if __name__ == "__main__":
    rng = np.random.default_rng(0)
    qkv = rng.standard_normal((B, S, D)).astype(np.float32)
    sc = 1.0 / np.sqrt(D)
    W_in = rng.uniform(-sc, sc, (D, 3 * D)).astype(np.float32)
    b_in = rng.uniform(-sc, sc, (3 * D,)).astype(np.float32)
    W_out = rng.uniform(-sc, sc, (D, D)).astype(np.float32)
    b_out = rng.uniform(-sc, sc, (D,)).astype(np.float32)
    got = kernel(qkv, W_in, b_in, W_out, b_out)
    print("kernel ran, output shape", got.shape)

